# revision 32
# baseline (speedup 1.0000x reference)
"""Multi-head attention (B=2, S=2048, D=2048, H=16, RoPE, causal) on 8 TRN2 cores.

Sharding: tensor-parallel over heads (2 heads/core) x batch as data.  Each core:
  phase 1: qkv projection for its 2 heads (both batches), RoPE fused into drain.
           qT,kT produced transposed [Dh, S]; v produced natural [S, Dh].
  phase 2: causal attention per (b,h) pair: s^T = kT.T @ qT blocks -> exp ->
           mask -> oT += v.T @ pT, row-sums l += ones.T @ pT (PSUM accum).
           Diagonal-trimmed (queries >= key only), exp batched 1024-wide over
           2-bank PSUM tiles, the two heads software-pipelined job-by-job so
           the PE never waits on exp latency.
  phase 3: partial out-proj: out_partial = sum_h diag(1/l_h) oT_h.T @ Wout_h.
Host sums the 8 partial outputs and adds b_out.

DMA discipline: one batched DMA per x chunk / weight group (3D access
patterns over host-rearranged layouts); weight DMAs ride the scalar-engine
HWDGE ring so they don't serialize behind x on the sync ring.  Output rows
are written with one 4KB-per-partition DMA per 128-token block.
"""

import numpy as np
import ml_dtypes

B, S, D = 2, 2048, 2048
H, DH = 16, 128
NCORES = 8
HPC = H // NCORES          # heads per core
T = B * S                  # 4096 tokens
SCALE = 1.0 / float(np.sqrt(DH))
ROPE_BASE = 10000.0

TC_N = T // 512            # 8 token chunks of 512 (phase 1)
KT_N = D // 128            # 16 contraction tiles
JB_N = S // 128            # 16 key blocks per sequence
IC_N = S // 512            # 4 query chunks per sequence
TT_N = S // 128            # 16 token tiles per batch (phase 3)
NC_N = D // 512            # 4 out-column chunks

# diagonal-band packing for the causal attention: per query chunk ic, key
# blocks [4ic, 4ic+4) are only needed for queries >= key.  d-th diag block
# covers query cols [128d, 512) of the chunk -> widths 512-128d.
DIAG_W = [512, 384, 256, 128]

_CACHE = {}


def _build_program():
    import concourse.bacc as bacc
    import concourse.mybir as mybir
    import concourse.tile as tile
    import concourse.bass as bass

    f32 = mybir.dt.float32
    bf16 = mybir.dt.bfloat16
    add = mybir.AluOpType.add
    mult = mybir.AluOpType.mult
    Exp = mybir.ActivationFunctionType.Exp
    Copy = mybir.ActivationFunctionType.Copy
    Ident = mybir.ActivationFunctionType.Identity
    PSUM = bass.MemorySpace.PSUM

    nc = bacc.Bacc("TRN2", target_bir_lowering=False, debug=False)

    # host-rearranged layouts: partition dim first, k-tile second
    xT = nc.dram_tensor("xT", [128, KT_N, T], bf16, kind="ExternalInput")
    wq = nc.dram_tensor("wq", [128, KT_N, HPC * DH], bf16, kind="ExternalInput")
    wk = nc.dram_tensor("wk", [128, KT_N, HPC * DH], bf16, kind="ExternalInput")
    wv = nc.dram_tensor("wv", [128, KT_N, HPC * DH], bf16, kind="ExternalInput")
    wo = nc.dram_tensor("wo", [128, HPC, D], bf16, kind="ExternalInput")
    bq = nc.dram_tensor("bq", [DH, HPC], f32, kind="ExternalInput")
    bk = nc.dram_tensor("bk", [DH, HPC], f32, kind="ExternalInput")
    bvb = nc.dram_tensor("bvb", [128, HPC * DH], f32, kind="ExternalInput")
    cos2 = nc.dram_tensor("cos2", [DH, S], bf16, kind="ExternalInput")
    sin2 = nc.dram_tensor("sin2", [DH, S], bf16, kind="ExternalInput")
    masks = nc.dram_tensor("masks", [DH, 128], bf16, kind="ExternalInput")
    out = nc.dram_tensor("out", [T, D], bf16, kind="ExternalOutput")

    with tile.TileContext(nc) as tc:
        with tc.tile_pool(name="persist", bufs=1) as pp:
            # --- resident weights/constants ---
            wq_sb = pp.tile([128, KT_N * 256], bf16, tag="wq_sb", name="wq_sb")
            wk_sb = pp.tile([128, KT_N * 256], bf16, tag="wk_sb", name="wk_sb")
            wv_sb = pp.tile([128, KT_N * 256], bf16, tag="wv_sb", name="wv_sb")
            wo_sb = pp.tile([128, HPC * D], bf16, tag="wo_sb", name="wo_sb")
            cos2_sb = pp.tile([DH, S], bf16, tag="cos2_sb", name="cos2_sb")
            sin2_sb = pp.tile([DH, S], bf16, tag="sin2_sb", name="sin2_sb")
            masks_sb = pp.tile([DH, 128], bf16, tag="masks_sb", name="masks_sb")
            bq_sb = pp.tile([DH, HPC], f32, tag="bq_sb", name="bq_sb")
            bk_sb = pp.tile([DH, HPC], f32, tag="bk_sb", name="bk_sb")
            bvb_sb = pp.tile([128, HPC * DH], f32, tag="bvb_sb", name="bvb_sb")
            # all-ones stationary: ones128.T @ pt replicates colsums to all
            # 128 PSUM partitions -> denominator tile needs no broadcast
            ones_sb = pp.tile([128, 128], bf16, tag="ones_sb", name="ones_sb")
            nc.vector.memset(ones_sb[:], 1.0)
            warm_sb = pp.tile([128, 512], bf16, tag="warm_sb", name="warm_sb")
            nc.vector.memset(warm_sb[:], 0.0)

            # HAM warmup: dummy matmuls while the first DMAs land, so the
            # PE clock-gate is already at 8/8 when real work starts.
            with tc.tile_pool(name="pswm", bufs=1, space=PSUM) as pswm:
                wmt = pswm.tile([128, 512], f32, tag="wmt", name="wmt")
                for i in range(14):
                    nc.tensor.matmul(wmt[:], ones_sb[:], warm_sb[:],
                                     start=True, stop=True)

            # --- per-(b,h) persistent tensors ---
            qT, kT, vN, oT = {}, {}, {}, {}
            for b in range(B):
                for h in range(HPC):
                    qT[b, h] = pp.tile([128, S], bf16, tag=f"qT{b}{h}", name=f"qT{b}{h}")
                    kT[b, h] = pp.tile([128, S], bf16, tag=f"kT{b}{h}", name=f"kT{b}{h}")
                    vN[b, h] = pp.tile([128, S], bf16, tag=f"vN{b}{h}", name=f"vN{b}{h}")
                    oT[b, h] = pp.tile([128, S], bf16, tag=f"oT{b}{h}", name=f"oT{b}{h}")

            # ================= phase 1: qkv projection =================
            with tc.tile_pool(name="xtp", bufs=3) as xtp, \
                 tc.tile_pool(name="ps_qk", bufs=5, space=PSUM) as ps_qk, \
                 tc.tile_pool(name="ps_v", bufs=3, space=PSUM) as ps_v, \
                 tc.tile_pool(name="rtp", bufs=4) as rtp:
                # chunk 0 split in 4 k-groups interleaved with the weights so
                # the PE can start ~1MB in; x rides the sync HWDGE ring, the
                # weights/tables the scalar ring (parallel transfer).
                xt0 = xtp.tile([128, KT_N * 512], bf16, tag="xt", name="xt0")
                # chunk-0 granularity ramps up: k-singles first so the very
                # first matmul only waits on ~200KB, then groups of 4.
                groups = [(0, 1), (1, 2), (2, 3), (3, 4), (4, 6), (6, 8),
                          (8, 10), (10, 12), (12, 14), (14, 16)]
                for ka, kb in groups:
                    # x at k-single granularity so each matmul's wait is as
                    # fine as possible; weights at the group granularity
                    for k1 in range(ka, kb):
                        nc.sync.dma_start(
                            xt0[:, k1 * 512:(k1 + 1) * 512], xT[:, k1:k1 + 1, 0:512])
                    nc.scalar.dma_start(
                        wq_sb[:, ka * 256:kb * 256], wq[:, ka:kb, :])
                    nc.scalar.dma_start(
                        wk_sb[:, ka * 256:kb * 256], wk[:, ka:kb, :])
                    # v weights ride the sync ring to balance early bandwidth
                    nc.sync.dma_start(
                        wv_sb[:, ka * 256:kb * 256], wv[:, ka:kb, :])
                nc.scalar.dma_start(cos2_sb[:], cos2[:])
                nc.scalar.dma_start(sin2_sb[:], sin2[:])
                nc.scalar.dma_start(bq_sb[:], bq[:])
                nc.scalar.dma_start(bk_sb[:], bk[:])
                nc.scalar.dma_start(bvb_sb[:], bvb[:])
                nc.scalar.dma_start(wo_sb[:], wo[:])
                nc.scalar.dma_start(masks_sb[:], masks[:])
                for tcn in range(TC_N):
                    b = tcn // 4
                    s0 = (tcn % 4) * 512
                    if tcn == 0:
                        xt = xt0
                    else:
                        xt = xtp.tile([128, KT_N * 512], bf16, tag="xt", name=f"xt{tcn}")
                        # chunks 1-2 split into k-groups so early-k matmuls
                        # start before the whole 2MB lands; later chunks are
                        # far enough ahead of compute for a single DMA
                        nsub = {1: 8, 2: 4, 3: 2}.get(tcn, 1)
                        ksz = KT_N // nsub
                        for g in range(nsub):
                            nc.sync.dma_start(
                                xt[:, g * ksz * 512:(g + 1) * ksz * 512],
                                xT[:, g * ksz:(g + 1) * ksz,
                                   tcn * 512:(tcn + 1) * 512])
                    # all 8 accumulation chains (4 q/k + 4 v) run k-major so
                    # each short v-LDWEIGHTS hides under a longer q/k stream
                    qk_tiles = []
                    for gi, (wsb, bias, dst) in enumerate(
                            ((wq_sb, bq_sb, qT), (wk_sb, bk_sb, kT))):
                        for h in range(HPC):
                            ps = ps_qk.tile([128, 512], f32, tag="psqk",
                                            name=f"psqk{tcn}{gi}{h}")
                            qk_tiles.append((ps, wsb, bias, dst, h))
                    pv = [ps_v.tile([128, 512], f32, tag="psv", name=f"psv{tcn}{hf}")
                          for hf in range(2)]
                    for k in range(KT_N):
                        # interleave the short v matmuls between the 512-wide
                        # q/k streams so every LDWEIGHTS hides under a stream
                        vjobs = [(hf, sub) for hf in range(2) for sub in range(2)]
                        for i, (ps, wsb, bias, dst, h) in enumerate(qk_tiles):
                            nc.tensor.matmul(
                                ps[:],
                                wsb[:, k * 256 + h * 128: k * 256 + (h + 1) * 128],
                                xt[:, k * 512:(k + 1) * 512],
                                start=(k == 0), stop=(k == KT_N - 1))
                            hf, sub = vjobs[i]
                            t_sub = hf * 2 + sub
                            nc.tensor.matmul(
                                pv[hf][:, sub * 256:(sub + 1) * 256],
                                xt[:, k * 512 + t_sub * 128: k * 512 + (t_sub + 1) * 128],
                                wv_sb[:, k * 256:(k + 1) * 256],
                                start=(k == 0 and sub == 0),
                                stop=(k == KT_N - 1 and sub == 1),
                                skip_group_check=True)
                    # v drains FIRST (frees the ps_v slots for the next
                    # chunk's early v matmuls): psum halves -> tiles + bias.
                    # On the LAST chunk the qk drains go first instead: the
                    # attention pool reuses the ps_qk banks, and batch-0's
                    # attention only waits on those banks, not on this data.
                    def v_drains():
                        for hf in range(2):
                            for sub in range(2):
                                t_sub = hf * 2 + sub
                                jblk = (tcn % 4) * 4 + t_sub
                                for h in range(HPC):
                                    nc.vector.tensor_tensor(
                                        vN[b, h][:, jblk * 128:(jblk + 1) * 128],
                                        pv[hf][:, sub * 256 + h * 128: sub * 256 + (h + 1) * 128],
                                        bvb_sb[:, h * 128:(h + 1) * 128], op=add)
                    if tcn < TC_N - 1:
                        v_drains()
                    for ps, wsb, bias, dst, h in qk_tiles:
                            qsb = rtp.tile([128, 512], bf16, tag="qsb", name=f"qsb{tcn}{h}{id(dst)%97}")
                            nc.scalar.activation(qsb[:], ps[:], Ident, bias=bias[:, h:h + 1])
                            # half-swapped copy (rotate_half) via SBUF->SBUF DMA:
                            # DVE ops can't cross partition boundaries.
                            qsw = rtp.tile([128, 512], bf16, tag="qsw", name=f"qsw{tcn}{h}")
                            nc.gpsimd.dma_start(qsw[0:64, :], qsb[64:128, :])
                            nc.gpsimd.dma_start(qsw[64:128, :], qsb[0:64, :])
                            t1 = rtp.tile([128, 512], bf16, tag="t1", name=f"t1_{tcn}{h}")
                            t2 = rtp.tile([128, 512], bf16, tag="t2", name=f"t2_{tcn}{h}")
                            nc.vector.tensor_tensor(
                                t1[:], qsb[:], cos2_sb[:, s0:s0 + 512], op=mult)
                            nc.vector.tensor_tensor(
                                t2[:], qsw[:], sin2_sb[:, s0:s0 + 512], op=mult)
                            nc.vector.tensor_tensor(
                                dst[b, h][:, s0:s0 + 512], t1[:], t2[:], op=add)
                    if tcn == TC_N - 1:
                        v_drains()

            # ================= phase 2: causal attention =================
            # Per (b, ic): one job per key block (512-wide scores -> exp ->
            # AV/rowsum), the two heads interleaved, with 4 independent
            # score PSUM banks so scores never throttle on exp completions,
            # and the AV matmuls lagging LAG jobs behind so neither the exp
            # latency nor the previous chunk's DVE drains stall the PE.
            with tc.tile_pool(name="ps_s", bufs=4, space=PSUM) as ps_s, \
                 tc.tile_pool(name="ps_o", bufs=2, space=PSUM) as ps_o, \
                 tc.tile_pool(name="ps_l", bufs=2, space=PSUM) as ps_l, \
                 tc.tile_pool(name="ptp", bufs=16) as ptp, \
                 tc.tile_pool(name="rrp", bufs=2) as rrp, \
                 tc.tile_pool(name="outp", bufs=4) as outp:
                for b in range(B):
                    # descending ic: the dense full-block units run first so
                    # the PE stays busy (HAM warm) right after phase 1
                    for ic in reversed(range(IC_N)):
                        pso, psl = {}, {}
                        for h in range(HPC):
                            pso[h] = ps_o.tile([128, 512], f32, tag="pso",
                                               name=f"pso{b}{h}{ic}")
                            psl[h] = ps_l.tile([128, 512], f32, tag="psl",
                                               name=f"psl{b}{h}{ic}")
                        # jobs: full key blocks, then the 4 diagonal blocks
                        # at trimmed widths (queries >= key only)
                        jobs = [(jb, 512) for jb in range(4 * ic)] + \
                               [(4 * ic + dd, DIAG_W[dd]) for dd in range(4)]
                        nj = len(jobs)

                        def emit_scores_exp(j, h):
                            jb, w = jobs[j]
                            pss = ps_s.tile([128, 512], f32, tag="pss",
                                            name=f"pss{b}{h}{ic}j{j}")
                            pt = ptp.tile([128, 512], bf16, tag="pt",
                                          name=f"pt{b}{h}{ic}j{j}")
                            nc.tensor.matmul(
                                pss[:, 0:w],
                                kT[b, h][:, jb * 128:(jb + 1) * 128],
                                qT[b, h][:, ic * 512 + 512 - w: (ic + 1) * 512],
                                start=True, stop=True)
                            nc.scalar.activation(pt[:, 0:w], pss[:, 0:w],
                                                 Exp, scale=SCALE)
                            if w < 512 or jb == 4 * ic:
                                # diagonal block: triangular mask on the first
                                # 128 query cols
                                nc.vector.tensor_tensor(
                                    pt[:, 0:128], pt[:, 0:128], masks_sb[:], op=mult)
                            return pt

                        def emit_avl(j, h, pt):
                            jb, w = jobs[j]
                            first, last = j == 0, j == nj - 1
                            nc.tensor.matmul(
                                pso[h][:, 512 - w:512],
                                vN[b, h][:, jb * 128:(jb + 1) * 128],
                                pt[:, 0:w],
                                start=first, stop=last, skip_group_check=True)
                            nc.tensor.matmul(
                                psl[h][:, 512 - w:512], ones_sb[:], pt[:, 0:w],
                                start=first, stop=last, skip_group_check=True)

                        LAG = min(6, nj)
                        pts = {}
                        for j in range(nj + LAG):
                            for h in range(HPC):
                                if j < nj:
                                    pts[j, h] = emit_scores_exp(j, h)
                                if j >= LAG:
                                    emit_avl(j - LAG, h, pts.pop((j - LAG, h)))
                                if j == nj + LAG - 1:
                                    # drain right after this head's last AV so
                                    # the DVE chain overlaps the other head's
                                    # remaining matmuls: oT = pso * (1/l)
                                    rr = rrp.tile([128, 512], f32, tag="rr",
                                                  name=f"rr{b}{h}{ic}")
                                    nc.vector.reciprocal_approx_fast(rr[:], psl[h][:])
                                    nc.vector.tensor_tensor(
                                        oT[b, h][:, ic * 512:(ic + 1) * 512],
                                        pso[h][:], rr[:], op=mult)

                # ============= phase 3: output projection =============
                # No pool transition: the projection PSUM tiles reuse the
                # attention pools' slots (same tag/shape) so the matmul
                # pairs chain straight into the freed banks; ACT/DVE
                # alternate the fp32 drains into a 4KB/partition staging
                # tile flushed with ONE dma per 128-token row block.
                pools3 = [(ps_s, "pss"), (ps_o, "pso"), (ps_l, "psl"),
                          (ps_s, "pss")]
                dcnt = 0
                for b in range(B):
                    for tt in range(TT_N):
                        osb = outp.tile([128, D], bf16, tag="osb", name=f"osb{b}{tt}")
                        for ncx in range(NC_N):
                            pool, ptag = pools3[ncx]
                            ps = pool.tile([128, 512], f32, tag=ptag,
                                           name=f"ps3{b}{tt}{ncx}")
                            nc.tensor.matmul(
                                ps[:],
                                oT[b, 0][:, tt * 128:(tt + 1) * 128],
                                wo_sb[:, 0 * D + ncx * 512: 0 * D + (ncx + 1) * 512],
                                start=True, stop=False)
                            nc.tensor.matmul(
                                ps[:],
                                oT[b, 1][:, tt * 128:(tt + 1) * 128],
                                wo_sb[:, 1 * D + ncx * 512: 1 * D + (ncx + 1) * 512],
                                start=False, stop=True)
                            if dcnt % 2 == 0:
                                nc.scalar.activation(
                                    osb[:, ncx * 512:(ncx + 1) * 512], ps[:], Copy)
                            else:
                                nc.vector.tensor_copy(
                                    osb[:, ncx * 512:(ncx + 1) * 512], ps[:])
                            dcnt += 1
                            last = (b == B - 1 and tt == TT_N - 1)
                            if last:
                                # final row block: per-chunk DMAs so the HBM
                                # write receipts pipeline with the drains
                                row0 = b * S + tt * 128
                                nc.sync.dma_start(
                                    out[row0:row0 + 128, ncx * 512:(ncx + 1) * 512],
                                    osb[:, ncx * 512:(ncx + 1) * 512])
                        if not last:
                            row0 = b * S + tt * 128
                            nc.sync.dma_start(out[row0:row0 + 128, :], osb[:])

    nc.compile()
    return nc


def _host_prep(x, w_qkv, b_qkv, w_out, b_out):
    """Build the 8 per-core input maps."""
    bf = ml_dtypes.bfloat16
    xT = np.ascontiguousarray(x.reshape(T, D).T).astype(bf)      # [D, T]
    # rearrange to [128, KT_N, T]: partition-major, k-tile second
    xTr = np.ascontiguousarray(xT.reshape(KT_N, 128, T).transpose(1, 0, 2))

    # RoPE tables: cos/sin [S, DH//2] -> stacked transposed [DH, S]
    inv_freq = 1.0 / (ROPE_BASE ** (np.arange(0, DH, 2, dtype=np.float32) / DH))
    t = np.arange(S, dtype=np.float32)
    freqs = np.outer(t, inv_freq)                       # [S, 64]
    cosT = np.cos(freqs).T.astype(np.float32)           # [64, S]
    sinT = np.sin(freqs).T.astype(np.float32)
    cos2 = np.concatenate([cosT, cosT], axis=0).astype(bf)      # [128, S]
    sin2 = np.concatenate([-sinT, sinT], axis=0).astype(bf)     # [128, S]

    # triangular causal mask for the 128x128 diagonal sub-blocks
    jj = np.arange(128)[:, None]
    ii = np.arange(128)[None, :]
    masks = (jj <= ii).astype(np.float32).astype(bf)            # [128, 128]

    def k_rearrange(w):                  # [D, M] -> [128, KT_N, M]
        return np.ascontiguousarray(
            w.reshape(KT_N, 128, w.shape[1]).transpose(1, 0, 2))

    in_maps = []
    for c in range(NCORES):
        h0 = c * HPC
        cols = slice(h0 * DH, (h0 + HPC) * DH)
        wq_c = w_qkv[:, cols].astype(bf)
        wk_c = w_qkv[:, D + h0 * DH: D + (h0 + HPC) * DH].astype(bf)
        wv_c = w_qkv[:, 2 * D + h0 * DH: 2 * D + (h0 + HPC) * DH].astype(bf)
        wo_c = w_out[cols, :].astype(bf)                         # [256, D]
        wo_r = np.ascontiguousarray(wo_c.reshape(HPC, 128, D).transpose(1, 0, 2))
        bq_c = b_qkv[cols].reshape(HPC, DH).T.astype(np.float32)          # [128, 2]
        bk_c = b_qkv[D + h0 * DH: D + (h0 + HPC) * DH].reshape(HPC, DH).T.astype(np.float32)
        bv_c = b_qkv[2 * D + h0 * DH: 2 * D + (h0 + HPC) * DH].astype(np.float32)
        bvb_c = np.broadcast_to(bv_c[None, :], (128, HPC * DH)).copy()
        in_maps.append({
            "xT": xTr, "wq": k_rearrange(wq_c), "wk": k_rearrange(wk_c),
            "wv": k_rearrange(wv_c), "wo": wo_r,
            "bq": np.ascontiguousarray(bq_c), "bk": np.ascontiguousarray(bk_c),
            "bvb": bvb_c, "cos2": cos2, "sin2": sin2, "masks": masks,
        })
    return in_maps


def _get_program():
    if "nc" not in _CACHE:
        _CACHE["nc"] = _build_program()
    return _CACHE["nc"]


def run_on_hw(in_maps, trace=False, **kw):
    from concourse.bass_utils import run_bass_kernel_spmd
    nc = _get_program()
    return run_bass_kernel_spmd(nc, in_maps, core_ids=list(range(NCORES)),
                                trace=trace, **kw)


def kernel(x, w_qkv, b_qkv, w_out, b_out):
    x = np.asarray(x, dtype=np.float32)
    w_qkv = np.asarray(w_qkv, dtype=np.float32)
    b_qkv = np.asarray(b_qkv, dtype=np.float32)
    w_out = np.asarray(w_out, dtype=np.float32)
    b_out = np.asarray(b_out, dtype=np.float32)

    in_maps = _host_prep(x, w_qkv, b_qkv, w_out, b_out)
    res = run_on_hw(in_maps)
    acc = np.zeros((T, D), dtype=np.float32)
    for c in range(NCORES):
        acc += res.results[c]["out"].astype(np.float32)
    acc += b_out[None, :]
    return acc.reshape(B, S, D)
